# revision 1
# baseline (speedup 1.0000x reference)
"""STEBitLinear Trainium2 kernel.

y[b,s,o] = sum_i x[b,s,i] * sign(w[o,i]) * scale[o, i//128]

Strategy: data-parallel over the flattened (b,s) dim across 8 NeuronCores
(weights/scales replicated, no collectives). Per core:
  - cast x shard to bf16, PE-transpose into a resident SBUF x^T
  - per 512-wide out-feature tile: build w_eff^T = (sign*scale)^T in bf16
    (fused cast+scale via per-partition tensor_scalar, then PE transpose)
  - 128x128x512 bf16 matmuls accumulating over K=4096 in PSUM (fp32)
"""

import sys

for _p in ("/opt/trn_rl_repo", "/opt/pypackages"):
    if _p not in sys.path:
        sys.path.append(_p)

import numpy as np

import concourse.bacc as bacc
import concourse.mybir as mybir
from concourse.bass_utils import run_bass_kernel_spmd
from concourse.masks import make_identity
from concourse.tile import TileContext

N_CORES = 8
B, S, IN_F, OUT_F = 4, 2048, 4096, 4096
GROUP = 128
M_FULL = B * S  # 8192


def build_program(M=M_FULL // N_CORES, K=IN_F, N=OUT_F, n_tile=512):
    """Emit the per-core Bass program (SPMD: same program on all cores)."""
    P = 128
    KT = K // P            # k tiles (contraction, partition dim)
    MT = M // P            # m tiles
    NT = N // n_tile       # out-feature tiles
    NSUB = n_tile // P     # 128-wide o sub-blocks per o tile
    IC = K // 512          # 512-wide i chunks per row-block
    G = K // GROUP         # scale groups along in_features
    NB = N // P            # o blocks of 128
    bf16 = mybir.dt.bfloat16
    f32 = mybir.dt.float32

    nc = bacc.Bacc("TRN2", target_bir_lowering=False, debug=False)
    x_d = nc.dram_tensor("x", [M, K], f32, kind="ExternalInput").ap()
    w_d = nc.dram_tensor("sw", [N, K], f32, kind="ExternalInput").ap()
    sc_d = nc.dram_tensor("sc", [N, G], f32, kind="ExternalInput").ap()
    y_d = nc.dram_tensor("y", [M, N], f32, kind="ExternalOutput").ap()

    with TileContext(nc) as tc:
        with (
            tc.tile_pool(name="consts", bufs=1) as consts,
            tc.tile_pool(name="xt_pool", bufs=1) as xt_pool,
            tc.tile_pool(name="wt_pool", bufs=2) as wt_pool,
            tc.tile_pool(name="load", bufs=6) as load_pool,
            tc.tile_pool(name="stage", bufs=4) as stage_pool,
            tc.tile_pool(name="ysb", bufs=3) as y_pool,
            tc.tile_pool(name="pst", bufs=4, space="PSUM") as psum_t,
            tc.tile_pool(name="psa", bufs=2, space="PSUM") as psum_a,
        ):
            ident = consts.tile([P, P], bf16)
            make_identity(nc, ident)

            # scales resident: sc_sb[p, ob*G + g] = scales[ob*128 + p, g]
            sc_sb = consts.tile([P, NB * G], f32)
            for ob in range(NB):
                nc.sync.dma_start(
                    out=sc_sb[:, ob * G:(ob + 1) * G],
                    in_=sc_d[ob * P:(ob + 1) * P, :],
                )

            # ---- phase 0: x^T resident (bf16), [P, KT * M] ----
            xT = xt_pool.tile([P, KT * M], bf16)
            xT_v = xT.rearrange("p (k m) -> p k m", k=KT)
            for mt in range(MT):
                for ic in range(IC):
                    xin = load_pool.tile([P, 512], f32, tag="xload")
                    nc.sync.dma_start(
                        out=xin,
                        in_=x_d[mt * P:(mt + 1) * P, ic * 512:(ic + 1) * 512],
                    )
                    xbf = stage_pool.tile([P, 512], bf16, tag="xcast")
                    nc.vector.tensor_copy(out=xbf, in_=xin)
                    pt = psum_t.tile([P, 512], bf16, tag="pt")
                    for g in range(4):
                        nc.tensor.transpose(
                            pt[:, g * P:(g + 1) * P],
                            xbf[:, g * P:(g + 1) * P],
                            ident,
                        )
                    pt_v = pt.rearrange("p (g c) -> p g c", g=4)
                    nc.vector.tensor_copy(
                        out=xT_v[:, ic * 4:(ic + 1) * 4, mt * P:(mt + 1) * P],
                        in_=pt_v,
                    )

            # ---- main loop over out-feature tiles ----
            for ot in range(NT):
                # build w_eff^T for this o tile: [P, KT * n_tile] bf16
                wT = wt_pool.tile([P, KT * n_tile], bf16, tag="wt")
                wT_v = wT.rearrange("p (k o) -> p k o", k=KT)
                for j in range(NSUB):
                    ob = ot * NSUB + j
                    for ic in range(IC):
                        win = load_pool.tile([P, 512], f32, tag="wload")
                        nc.sync.dma_start(
                            out=win,
                            in_=w_d[ob * P:(ob + 1) * P, ic * 512:(ic + 1) * 512],
                        )
                        wbf = stage_pool.tile([P, 512], bf16, tag="wcast")
                        for g in range(4):
                            gk = ic * 4 + g
                            nc.vector.tensor_scalar_mul(
                                out=wbf[:, g * P:(g + 1) * P],
                                in0=win[:, g * P:(g + 1) * P],
                                scalar1=sc_sb[:, ob * G + gk:ob * G + gk + 1],
                            )
                        pt = psum_t.tile([P, 512], bf16, tag="pt")
                        for g in range(4):
                            nc.tensor.transpose(
                                pt[:, g * P:(g + 1) * P],
                                wbf[:, g * P:(g + 1) * P],
                                ident,
                            )
                        pt_v = pt.rearrange("p (g c) -> p g c", g=4)
                        nc.vector.tensor_copy(
                            out=wT_v[:, ic * 4:(ic + 1) * 4, j * P:(j + 1) * P],
                            in_=pt_v,
                        )

                # matmuls: for each m tile accumulate over all k tiles
                for mt in range(MT):
                    acc = psum_a.tile([P, n_tile], f32, tag="acc")
                    for k in range(KT):
                        nc.tensor.matmul(
                            acc,
                            xT_v[:, k, mt * P:(mt + 1) * P],
                            wT_v[:, k],
                            start=(k == 0),
                            stop=(k == KT - 1),
                        )
                    ysb = y_pool.tile([P, n_tile], f32, tag="ysb")
                    nc.vector.tensor_copy(out=ysb, in_=acc)
                    nc.sync.dma_start(
                        out=y_d[mt * P:(mt + 1) * P, ot * n_tile:(ot + 1) * n_tile],
                        in_=ysb,
                    )

    nc.compile()
    return nc


_nc_cache = {}


def _get_nc(key, **kw):
    if key not in _nc_cache:
        _nc_cache[key] = build_program(**kw)
    return _nc_cache[key]


def kernel(x: np.ndarray, sign_weights: np.ndarray, scales: np.ndarray) -> np.ndarray:
    nc = _get_nc("full")
    M_SH = M_FULL // N_CORES
    xf = np.ascontiguousarray(x.reshape(M_FULL, IN_F).astype(np.float32, copy=False))
    sw = np.ascontiguousarray(sign_weights.astype(np.float32, copy=False))
    sc = np.ascontiguousarray(scales.reshape(OUT_F, IN_F // GROUP))
    in_maps = [
        {"x": xf[c * M_SH:(c + 1) * M_SH], "sw": sw, "sc": sc}
        for c in range(N_CORES)
    ]
    res = run_bass_kernel_spmd(nc, in_maps, core_ids=list(range(N_CORES)))
    y = np.concatenate([res.results[c]["y"] for c in range(N_CORES)], axis=0)
    return y.reshape(B, S, OUT_F)
